# revision 20
# baseline (speedup 1.0000x reference)
"""DGC (GCN-style message passing) Trainium2 kernel, 8 NeuronCores. v2'.

Strategy:
  - Nodes dst-sharded across 8 cores (12500 each, padded to 12544 slots/core).
  - Self-loop term handled analytically per node (h *= 1-2*eps*dinv^2), so
    edge slots exclude self loops (~6% fewer slots).
  - Per iteration, each core gathers h[src] rows (fp16, 256B rows) from 4
    chunk-partitioned HBM tables via dma_gather (int16 indices; chunk
    tables sized under the 32767-row int16 limit).
  - Scatter one-hot matrices (norm-scaled fp8, zero on pad slots) are
    host-precomputed per edge slot and streamed from DRAM each iteration,
    feeding mixed fp8 x fp16 scatter matmuls (HBM bandwidth is far from
    the wall; DVE on-chip builds measured slower than the Pool-engine
    gather emission that actually binds).
  - Scatter-accumulate via per-window matmul chains in PSUM (fp8 one-hot
    lhsT x fp16 payload), update h32 (incl. the analytic self-loop scale),
    flush stage windows to the bounce buffer, and AllGather each chunk as
    soon as its windows are flushed (packed 64-col payload, repacked
    locally into 256B-row gather tables at iteration end). Double-buffered
    tables (parity per iteration). No flush/AllGather after the final
    iteration.
  - Embedding uses a single DMA-transpose load of x^T; readout bounces
    tanh(h) through DRAM for a DMA-transpose and a feature-contracted
    matmul into the transposed output.
"""
import os
import sys
import numpy as np

import concourse.bass as bass
import concourse.mybir as mybir
import concourse.tile as tile
from concourse import bacc, bass_utils

# Problem constants (hardcoded per spec nn_DGC_4475355922586)
N = 100000
IN_DIM = 128
HID = 64
OUT_DIM = 64
EPS = 0.1
ITERS = 4

NC_ = 8
SH = 12500          # real nodes per core
PADSH = 12544       # padded slots per core (98 * 128)
WIN = 128
WPC = PADSH // WIN  # 98 windows per core
SWIN = 7            # windows per stage
NS = (WPC + SWIN - 1) // SWIN  # 14 stages

QB = [0, 28, 56, 84, 98]   # chunk boundaries (windows)
NCH = len(QB) - 1
RPC = [(QB[c + 1] - QB[c]) * WIN for c in range(NCH)]  # table rows/core/chunk

dt = mybir.dt

LAST_RESULTS = {}


def _ensure_ntff_hook():
    """Provide antenv.axon_hooks (missing in this image) so trace=True works."""
    try:
        import antenv.axon_hooks  # noqa: F401
        return
    except ImportError:
        pass
    import types

    import antenv

    mod = types.ModuleType("antenv.axon_hooks")
    _h = [None]
    mod.set_axon_ntff_profile_hook = lambda hook: _h.__setitem__(0, hook)
    mod.get_axon_ntff_profile_hook = lambda: _h[0]
    sys.modules["antenv.axon_hooks"] = mod
    antenv.axon_hooks = mod
    try:
        from trn_agent_boot.trn_boot import _ntff_profile_via_ctypes

        mod.set_axon_ntff_profile_hook(
            _ntff_profile_via_ctypes("/opt/axon/libaxon_pjrt.so")
        )
    except Exception:
        pass


def _preprocess(x, edge_index):
    src = edge_index[0].astype(np.int64)
    dst = edge_index[1].astype(np.int64)
    # degrees include the self loop (fill value 2.0, improved=True)
    deg = np.bincount(dst, minlength=N).astype(np.float64) + 2.0
    dinv = (1.0 / np.sqrt(deg)).astype(np.float32)
    norm = (dinv[src] * dinv[dst]).astype(np.float16)

    core = dst // SH
    dloc = dst % SH
    wid = dloc // WIN
    dstloc = (dloc % WIN).astype(np.int16)

    score = src // SH
    sloc = src % SH
    w_src = sloc // WIN
    spos = sloc % WIN
    qb_arr = np.asarray(QB)
    ch = np.searchsorted(qb_arr, w_src, side="right") - 1
    rpc_arr = np.asarray(RPC)
    # chunk-local table row (always < 8*RPC[ch] <= 28672, int16-safe)
    gidx = (score * rpc_arr[ch] + (w_src - qb_arr[ch]) * WIN + spos).astype(np.int16)

    # group ordinal in (stage, chunk, window) order
    group_of = np.zeros((WPC, NCH), np.int64)
    g = 0
    for s in range(NS):
        for c4 in range(NCH):
            for w_ in range(s * SWIN, min(WPC, (s + 1) * SWIN)):
                group_of[w_, c4] = g
                g += 1
    NG = g
    gid = group_of[wid, ch]

    counts = np.bincount(core * NG + gid, minlength=NC_ * NG).reshape(NC_, NG)
    cap = ((counts.max(0) + 127) // 128) * 128  # per-group slot cap (mult of 128)
    starts = np.zeros(NG + 1, np.int64)
    np.cumsum(cap, out=starts[1:])
    T = int(starts[-1])

    per_core = []
    for c in range(NC_):
        m = core == c
        gi = gid[m]
        order = np.argsort(gi, kind="stable")
        gis = gi[order]
        first = np.searchsorted(gis, np.arange(NG))
        posin = np.arange(gis.size) - first[gis]
        pos = starts[gis] + posin

        g_all = np.zeros(T, np.int16)
        g_all[pos] = gidx[m][order]
        # dma_gather index format: 16-wrapped, replicated to 128 partitions
        gidx_fmt = np.tile(np.ascontiguousarray(g_all.reshape(-1, 16).T), (8, 1))

        # scatter matrix: S[slot, c] = norm_e for c == dstloc_e, 0 elsewhere
        # (fp8 e4m3 norms: ~3% weight quantization, well within the 2e-2 gate)
        s_flat = np.zeros((T, WIN), mybir.dt.np(mybir.dt.float8e4))
        s_flat[pos, dstloc[m][order]] = norm[m][order]
        # SBUF-ready layout [128 partitions, T//128 tiles, 128 cols]
        s_fmt = np.ascontiguousarray(
            s_flat.reshape(T // 128, 128, WIN).transpose(1, 0, 2)
        )

        x_sh = np.zeros((PADSH, IN_DIM), np.float16)
        x_sh[:SH] = x[c * SH : (c + 1) * SH].astype(np.float16)

        # self-loop scale per node: h *= 1 - eps*2*dinv^2 (1.0 on pad rows)
        ss = np.ones(PADSH, np.float32)
        ss[:SH] = 1.0 - EPS * 2.0 * dinv[c * SH : (c + 1) * SH] ** 2
        ss_fmt = np.ascontiguousarray(ss.reshape(WPC, 128).T)  # [128, WPC]

        per_core.append(
            dict(x_sh=x_sh, gidx=gidx_fmt, smat=s_fmt, sscale=ss_fmt)
        )

    struct = dict(T=T, cap=cap, starts=starts, group_of=group_of, NG=NG)
    return per_core, struct


def _build(struct):
    cap = struct["cap"]
    starts = struct["starts"]
    group_of = struct["group_of"]
    T = struct["T"]

    # per-stage geometry
    stage_windows = [list(range(s * SWIN, min(WPC, (s + 1) * SWIN))) for s in range(NS)]
    stage_start = []
    stage_end = []
    call_info = []  # [s][c] = (slot_start, n_slots)
    for s in range(NS):
        ws = stage_windows[s]
        s0 = int(starts[group_of[ws[0], 0]])
        cinfo = []
        for c4 in range(NCH):
            g0 = group_of[ws[0], c4]
            g1 = group_of[ws[-1], c4]
            cinfo.append((int(starts[g0]), int(starts[g1 + 1] - starts[g0])))
        s1 = int(starts[group_of[ws[-1], NCH - 1] + 1])
        stage_start.append(s0)
        stage_end.append(s1)
        call_info.append(cinfo)
    SSmax = max(stage_end[s] - stage_start[s] for s in range(NS))
    KMAX = SSmax // 128

    # stage after which chunk q's windows are all flushed
    q_done_stage = [(QB[q + 1] - 1) // SWIN for q in range(NCH)]

    nc = bacc.Bacc(
        "TRN2",
        target_bir_lowering=False,
        debug=False,
        num_devices=NC_,
        num_swdge_queues=4,
    )

    x_in = nc.dram_tensor("x_sh", [PADSH, IN_DIM], dt.float16, kind="ExternalInput")
    gidx_in = nc.dram_tensor("gidx", [128, T // 16], dt.int16, kind="ExternalInput")
    smat_in = nc.dram_tensor("smat", [128, T // 128, WIN], dt.float8e4, kind="ExternalInput")
    sscale_in = nc.dram_tensor("sscale", [128, WPC], dt.float32, kind="ExternalInput")
    embw_in = nc.dram_tensor("embw", [IN_DIM, HID], dt.float16, kind="ExternalInput")
    embb_in = nc.dram_tensor("embb", [128, HID], dt.float32, kind="ExternalInput")
    row_in = nc.dram_tensor("row", [HID, OUT_DIM], dt.float16, kind="ExternalInput")
    rob_in = nc.dram_tensor("rob", [OUT_DIM, 1], dt.float32, kind="ExternalInput")
    outT = nc.dram_tensor("outT", [OUT_DIM, PADSH], dt.float32, kind="ExternalOutput")
    th_dram = nc.dram_tensor("th_dram", [PADSH, 128], dt.float16, kind="Internal")
    # packed AllGather landing tables (64-col) + double-buffered 256B-row
    # gather tables (parity alternates per iteration; repacked locally)
    ht64 = [
        nc.dram_tensor(
            f"ht64_{q}", [NC_ * RPC[q], HID], dt.float16,
            kind="Internal", addr_space="Shared",
        )
        for q in range(NCH)
    ]
    ht = [
        [
            nc.dram_tensor(
                f"ht{q}_{pr}", [NC_ * RPC[q], 128], dt.float16,
                kind="Internal",
            )
            for pr in range(2)
        ]
        for q in range(NCH)
    ]

    AOT = mybir.AluOpType

    with tile.TileContext(nc) as tc:
        with tc.tile_pool(name="const", bufs=1) as cp, \
             tc.tile_pool(name="dram", bufs=1, space="DRAM") as dp:
            embw_sb = cp.tile([IN_DIM, HID], dt.float16)
            embb_sb = cp.tile([128, HID], dt.float32)
            row_sb = cp.tile([HID, OUT_DIM], dt.float16)
            rob_sb = cp.tile([OUT_DIM, 1], dt.float32)
            sscale_sb = cp.tile([128, WPC], dt.float32)
            h32 = cp.tile([128, WPC, HID], dt.float32)

            nc.sync.dma_start(embw_sb[:], embw_in[:])
            nc.sync.dma_start(embb_sb[:], embb_in[:])
            nc.sync.dma_start(row_sb[:], row_in[:])
            nc.sync.dma_start(rob_sb[:], rob_in[:])
            nc.sync.dma_start(sscale_sb[:], sscale_in[:])

            bq = [
                dp.tile([RPC[q], HID], dt.float16, tag=f"bq{q}", name=f"bq{q}")
                for q in range(NCH)
            ]
            bvq = [
                bq[q][:].rearrange("(W p) f -> p W f", p=128) for q in range(NCH)
            ]  # [128, qw[q], HID]

            def stage_flush(s, h16_pool):
                """cast h32 stage windows to fp16 and stage into bounce."""
                ws = stage_windows[s]
                h16 = h16_pool.tile([128, SWIN, HID], dt.float16, tag="h16")
                nc.scalar.activation(
                    h16[:, : len(ws), :],
                    h32[:, ws[0] : ws[0] + len(ws), :],
                    mybir.ActivationFunctionType.Copy,
                )
                w0, w1 = ws[0], ws[-1] + 1
                for q in range(NCH):
                    a = max(w0, QB[q])
                    b = min(w1, QB[q + 1])
                    if a >= b:
                        continue
                    nc.sync.dma_start(
                        bvq[q][:, a - QB[q] : b - QB[q], :],
                        h16[:, a - w0 : b - w0, :],
                    )

            def allgather(q, parity):
                nc.gpsimd.collective_compute(
                    "AllGather",
                    AOT.bypass,
                    replica_groups=[list(range(NC_))],
                    ins=[bq[q][:].opt()],
                    outs=[ht64[q][:].opt()],
                )

            def repack(q, parity):
                # local repack of the packed AG landing table into the
                # 256B-row gather table; on the scalar-engine HWDGE ring so
                # it neither occupies the Pool engine nor its SWDGE rings.
                # Split to stay under the 16384-descriptor-per-AP DMA limit.
                rows = NC_ * RPC[q]
                half = 0
                while half < rows:
                    n = min(15360, rows - half)
                    nc.scalar.dma_start(
                        ht[q][parity][half : half + n, 0:HID],
                        ht64[q][half : half + n, :],
                    )
                    half += n

            # ---------- embedding: h0 = x @ emb_w + emb_b ----------
            with tc.tile_pool(name="embp", bufs=1) as ep, \
                 tc.tile_pool(name="h16p", bufs=2) as cp_h16, \
                 tc.tile_pool(name="embps", bufs=2, space="PSUM") as eps:
                xT = ep.tile([128, PADSH], dt.float16)
                nc.sync.dma_start_transpose(xT[:], x_in[:])
                for w_ in range(WPC):
                    hps = eps.tile([128, HID], dt.float32, tag="hps")
                    nc.tensor.matmul(
                        hps[:], xT[:, w_ * 128 : (w_ + 1) * 128], embw_sb[:],
                        start=True, stop=True,
                    )
                    nc.vector.tensor_tensor(h32[:, w_, :], hps[:], embb_sb[:], AOT.add)
                for s in range(NS):
                    stage_flush(s, cp_h16)
                    for q in range(NCH):
                        if q_done_stage[q] == s:
                            allgather(q, 0)
                for q in range(NCH):
                    repack(q, 0)

            # ---------- 4 message-passing iterations ----------
            with tc.tile_pool(name="mb", bufs=3) as mp, \
                 tc.tile_pool(name="sb", bufs=3) as sp_s, \
                 tc.tile_pool(name="ixp", bufs=3) as ixp, \
                 tc.tile_pool(name="h16p2", bufs=2) as cp_h16, \
                 tc.tile_pool(name="wps", bufs=4, space="PSUM") as wps:
                for it in range(ITERS):
                    rp = it % 2        # read parity
                    wp = (it + 1) % 2  # write parity
                    last = it == ITERS - 1
                    for s in range(NS):
                        ws = stage_windows[s]
                        sbase = stage_start[s]
                        n_slots = stage_end[s] - sbase
                        n_tiles = n_slots // 128
                        ix = ixp.tile([128, SSmax // 16], dt.int16, tag="ix")
                        ssb = sp_s.tile([128, KMAX, WIN], dt.float8e4, tag="ssb")
                        nc.sync.dma_start(
                            ix[:, : n_slots // 16],
                            gidx_in[:, sbase // 16 : stage_end[s] // 16],
                        )
                        nc.sync.dma_start(
                            ssb[:, :n_tiles, :],
                            smat_in[:, sbase // 128 : stage_end[s] // 128, :],
                        )
                        mbuf = mp.tile([128, KMAX, 128], dt.float16, tag="mb")
                        for c4 in range(NCH):
                            c_start, c_n = call_info[s][c4]
                            if c_n == 0:
                                continue
                            moff = (c_start - sbase) // 128
                            nc.gpsimd.dma_gather(
                                out_ap=mbuf[:, moff : moff + c_n // 128, :],
                                in_ap=ht[c4][rp][:],
                                idxs_ap=ix[
                                    :,
                                    (c_start - sbase) // 16 : (c_start - sbase) // 16
                                    + c_n // 16,
                                ],
                                num_idxs=c_n,
                                num_idxs_reg=c_n,
                                elem_size=128,
                                single_packet=False,
                                queue_num=c4,
                            )
                        ps = wps.tile([128, SWIN * HID], dt.float32, tag="ps")
                        for wl, w_ in enumerate(ws):
                            tl = []
                            for c4 in range(NCH):
                                g = group_of[w_, c4]
                                gt0 = int(starts[g]) // 128
                                for t in range(int(cap[g]) // 128):
                                    tl.append(gt0 + t - sbase // 128)
                            for r, mt in enumerate(tl):
                                nc.tensor.matmul(
                                    ps[:, wl * HID : (wl + 1) * HID],
                                    ssb[:, mt, :],
                                    mbuf[:, mt, 0:HID],
                                    start=(r == 0),
                                    stop=(r == len(tl) - 1),
                                )
                        for wl, w_ in enumerate(ws):
                            nc.vector.tensor_scalar(
                                h32[:, w_, :], h32[:, w_, :],
                                sscale_sb[:, w_ : w_ + 1], None, AOT.mult,
                            )
                            nc.vector.scalar_tensor_tensor(
                                h32[:, w_, :],
                                ps[:, wl * HID : (wl + 1) * HID],
                                -EPS,
                                h32[:, w_, :],
                                AOT.mult,
                                AOT.add,
                            )
                        if not last:
                            stage_flush(s, cp_h16)
                            for q in range(NCH):
                                if q_done_stage[q] == s:
                                    allgather(q, wp)
                    if not last:
                        for q in range(NCH):
                            repack(q, wp)

            # ---------- readout: out = tanh(h) @ ro_w + ro_b ----------
            with tc.tile_pool(name="rp", bufs=1) as rp_, \
                 tc.tile_pool(name="rob_p", bufs=2) as rp2, \
                 tc.tile_pool(name="rps", bufs=2, space="PSUM") as rps:
                th = rp_.tile([128, WPC, 128], dt.float16)
                nc.vector.memset(th[:], 0.0)
                nc.scalar.activation(
                    th[:, :, 0:HID], h32[:], mybir.ActivationFunctionType.Tanh
                )
                thd_view = th_dram[:].rearrange("(W p) f -> p W f", p=128)
                nc.sync.dma_start(thd_view, th[:])
                thT = rp_.tile([128, PADSH], dt.float16)
                nc.sync.dma_start_transpose(thT[:], th_dram[:])
                c0 = 0
                while c0 < PADSH:
                    cs = min(512, PADSH - c0)
                    ops = rps.tile([OUT_DIM, 512], dt.float32, tag="ops")
                    nc.tensor.matmul(
                        ops[:, :cs], row_sb[:], thT[0:HID, c0 : c0 + cs],
                        start=True, stop=True,
                    )
                    osb = rp2.tile([OUT_DIM, 512], dt.float32, tag="osb")
                    nc.vector.tensor_scalar(
                        osb[:, :cs], ops[:, :cs], rob_sb[:, 0:1], None, AOT.add
                    )
                    nc.sync.dma_start(outT[:, c0 : c0 + cs], osb[:, :cs])
                    c0 += cs

    nc.compile()
    return nc


def kernel(x, edge_index, emb_w, emb_b, ro_w, ro_b):
    x = np.asarray(x)
    edge_index = np.asarray(edge_index)
    per_core, struct = _preprocess(x, edge_index)

    embw_np = np.asarray(emb_w).astype(np.float16)
    embb_np = np.tile(np.asarray(emb_b).astype(np.float32)[None, :], (128, 1))
    row_np = np.asarray(ro_w).astype(np.float16)
    rob_np = np.asarray(ro_b).astype(np.float32)[:, None]

    nc = _build(struct)

    in_maps = []
    for c in range(NC_):
        pc = per_core[c]
        in_maps.append(
            dict(
                x_sh=pc["x_sh"],
                gidx=pc["gidx"],
                smat=pc["smat"],
                sscale=pc["sscale"],
                embw=embw_np,
                embb=embb_np,
                row=row_np,
                rob=rob_np,
            )
        )

    trace = bool(int(os.environ.get("KERNEL_TRACE", "0")))
    if trace:
        _ensure_ntff_hook()
    res = bass_utils.run_bass_kernel_spmd(
        nc, in_maps, core_ids=list(range(NC_)), trace=trace
    )
    LAST_RESULTS["res"] = res

    out = np.empty((N, OUT_DIM), np.float32)
    for c in range(NC_):
        out[c * SH : (c + 1) * SH] = res.results[c]["outT"].T[:SH]
    return out
